# revision 13
# baseline (speedup 1.0000x reference)
"""GNN message-passing kernel for Trainium2, 8 NeuronCores.

Sharding: destination-node partition; 8 cores x 49 windows of <=128 nodes.
Host precomputes node embeddings (h0), per-window edge slot layouts (edges
split by source table half for int16 dma_gather indices, ragged per-window
tile counts), edge-attr columns and 0/1 indicator tiles (bf16).

Table rows are laid out as [all cores' windows 0-21 | all cores' windows
22-48] so the next-layer table is published with TWO AllGathers, the first
overlapping the tail of pass B. Per layer on each core:
  - one dma_gather per table half per window (batched h[src] gather with
    exact per-core row counts via runtime registers; trailing -1 idxs)
  - e = eat @ [We;be] on PE (bf16), h-add via identity matmul into PSUM,
    msg = relu(.) split between Scalar and Vector engines (bf16 out)
  - segment-sum via host-precomputed indicator matmul into PSUM
  - own-h kept SBUF-resident (bf16); z = (1+eps)*h + agg on DVE
  - MLP1 (bf16) with zT built by DMA-transpose, BN batch stats via
    accum_out + AllReduce, BN-apply+ReLU fused on ScalarE, MLP2 (bf16)
    with b2 folded into PSUM via a K=1 ones-row matmul, table published
    via split bf16 AllGather
"""
import sys
import numpy as np
import ml_dtypes

sys.path.insert(0, "/opt/trn_rl_repo")

import concourse.bass as bass
import concourse.bacc as bacc
import concourse.mybir as mybir
import concourse.tile as tile
from concourse import bass_utils
from concourse.masks import make_identity

F32 = mybir.dt.float32
BF16 = mybir.dt.bfloat16
I32 = mybir.dt.int32
I16 = mybir.dt.int16
OP = mybir.AluOpType
AF = mybir.ActivationFunctionType
BF = ml_dtypes.bfloat16

N_NODES = 50000
N_EDGES = 300000
EMB = 256
HID = 512
L = 5
NCORES = 8
WPC = 49                      # windows per core
BINS = NCORES * WPC           # 392
ROWS_PC = WPC * 128           # 6272 padded rows per core
WLO = 22                      # windows in the first table half
LO_PC = WLO * 128             # rows per core in half 1
HI_PC = (WPC - WLO) * 128     # rows per core in half 2
ROWS_LO = NCORES * LO_PC      # int16-safe gather range
BN_EPS = 1e-5

_cache = {}


def _build(key):
    """Build + compile the SPMD program. key = (TLs, THs) per-window slots."""
    if key in _cache:
        return _cache[key]
    TLs, THs = key
    TLMAX, THMAX = max(TLs), max(THs)
    tpws = [(TLs[w] + THs[w]) // 128 for w in range(WPC)]
    TPWMAX = max(tpws)
    nc = bacc.Bacc("TRN2", target_bir_lowering=False, debug=False,
                   num_devices=NCORES, num_swdge_queues=4)
    d = {}
    d["h0b"] = nc.dram_tensor("h0b", [WPC, 128, EMB], BF16, kind="ExternalInput").ap()
    d["idxlo"] = nc.dram_tensor("idxlo", [WPC, 128, TLMAX // 16], I16, kind="ExternalInput").ap()
    d["idxhi"] = nc.dram_tensor("idxhi", [WPC, 128, THMAX // 16], I16, kind="ExternalInput").ap()
    d["cnts"] = nc.dram_tensor("cnts", [1, 2 * WPC], I32, kind="ExternalInput").ap()
    d["eattr"] = nc.dram_tensor("eattr", [WPC, 3, TPWMAX * 128], BF16, kind="ExternalInput").ap()
    d["indw"] = nc.dram_tensor("indw", [WPC, 128, TPWMAX * 128], BF16, kind="ExternalInput").ap()
    d["maskpw"] = nc.dram_tensor("maskpw", [128, WPC], F32, kind="ExternalInput").ap()
    d["welb"] = nc.dram_tensor("welb", [L, 3, EMB], BF16, kind="ExternalInput").ap()
    d["w1"] = nc.dram_tensor("w1", [L, EMB, HID], BF16, kind="ExternalInput").ap()
    d["w2"] = nc.dram_tensor("w2", [L, HID, EMB], BF16, kind="ExternalInput").ap()
    d["gamma"] = nc.dram_tensor("gamma", [L, HID, 1], F32, kind="ExternalInput").ap()
    d["beta"] = nc.dram_tensor("beta", [L, HID, 1], F32, kind="ExternalInput").ap()
    d["b2b"] = nc.dram_tensor("b2b", [L, 1, EMB], BF16, kind="ExternalInput").ap()
    d["epsr"] = nc.dram_tensor("epsr", [L, 128, 1], F32, kind="ExternalInput").ap()
    out_ext = nc.dram_tensor("out", [ROWS_PC, EMB], F32, kind="ExternalOutput").ap()

    chunks = [list(range(c * 4, min(c * 4 + 4, WPC))) for c in range((WPC + 3) // 4)]
    # first chunk index by which windows 0..WLO-1 are all written
    ci_lo = min(ci for ci, ch in enumerate(chunks) if ch[-1] >= WLO - 1)

    with tile.TileContext(nc) as tc:
        with tc.tile_pool(name="const", bufs=1) as cpool, \
             tc.tile_pool(name="wts", bufs=2) as wts, \
             tc.tile_pool(name="meta", bufs=4) as mpool, \
             tc.tile_pool(name="work", bufs=4) as wpool, \
             tc.tile_pool(name="resid", bufs=1) as rpool, \
             tc.tile_pool(name="stat", bufs=1) as spool, \
             tc.tile_pool(name="psA", bufs=2, space="PSUM") as psA, \
             tc.tile_pool(name="psE", bufs=2, space="PSUM") as psE, \
             tc.tile_pool(name="psY", bufs=2, space="PSUM") as psY, \
             tc.tile_pool(name="psH", bufs=2, space="PSUM") as psH, \
             tc.tile_pool(name="dram", bufs=1, space="DRAM") as dpool:

            # ---- constants ----
            ident = cpool.tile([128, 128], F32, name="ident")
            make_identity(nc, ident[:])
            identb = cpool.tile([128, 128], BF16, name="identb")
            nc.vector.tensor_copy(out=identb[:], in_=ident[:])
            maskw = cpool.tile([128, WPC], F32, name="maskw")
            nc.sync.dma_start(out=maskw[:], in_=d["maskpw"][:])
            ones1 = cpool.tile([1, 128], BF16, name="ones1")
            nc.gpsimd.memset(ones1[:], 1.0)
            cntt = cpool.tile([1, 2 * WPC], I32, name="cntt")
            nc.sync.dma_start(out=cntt[:], in_=d["cnts"][:])

            # ---- DRAM scratch ----
            cc_in = dpool.tile([ROWS_PC, EMB], BF16, name="cc_in")
            tabs_lo = [dpool.tile([ROWS_LO, EMB], BF16, name=f"tabl{l}",
                                  addr_space="Shared") for l in range(L)]
            tabs_hi = [dpool.tile([NCORES * ROWS_PC - ROWS_LO, EMB], BF16,
                                  name=f"tabh{l}",
                                  addr_space="Shared") for l in range(L)]
            stats_in = dpool.tile([HID, 2], F32, name="stats_in")
            stats_outs = [dpool.tile([HID, 2], F32, name=f"stats_out{l}",
                                     addr_space="Shared") for l in range(L)]

            # ---- gather-count register ring (reused; avoids reg pressure) ----
            cnt_regs = [nc.gpsimd.alloc_register(f"cntreg{i}") for i in range(4)]

            # ---- residents ----
            y1t = [rpool.tile([128, ROWS_PC], BF16, name=f"y1t{m}") for m in range(4)]
            zT = [rpool.tile([128, 512], BF16, name=f"zT{k}", bufs=2) for k in range(2)]
            scr = rpool.tile([128, 512], BF16, name="scr")
            ownh = rpool.tile([128, WPC * EMB], BF16, name="ownh")
            gbufs = [rpool.tile([128, TPWMAX, EMB], BF16, name=f"gb{i}")
                     for i in range(4)]

            def prologue():
                # own-h resident from host h0 (bf16)
                for w in range(WPC):
                    nc.sync.dma_start(out=ownh[:, w * EMB:(w + 1) * EMB],
                                      in_=d["h0b"][w])
                # seed cc_in (internal, collective-legal) from the host h0
                for w in range(0, WPC, 7):
                    hb = wpool.tile([128, 7 * EMB], BF16, name="hb", tag="hb")
                    nw = min(7, WPC - w)
                    rows = nw * 128
                    nc.sync.dma_start(
                        out=hb[:, :rows * EMB // 128],
                        in_=d["h0b"][w:w + nw])
                    nc.sync.dma_start(
                        out=cc_in[w * 128:w * 128 + rows, :],
                        in_=hb[:, :rows * EMB // 128])
                # scrub gather buffers (junk slots must be finite: 0*NaN=NaN)
                for gb in gbufs:
                    nc.gpsimd.memset(gb[:], 0.0)
                nc.gpsimd.collective_compute(
                    "AllGather", OP.bypass,
                    replica_groups=[list(range(NCORES))],
                    ins=[cc_in[0:LO_PC, :]], outs=[tabs_lo[0][:]])
                nc.gpsimd.collective_compute(
                    "AllGather", OP.bypass,
                    replica_groups=[list(range(NCORES))],
                    ins=[cc_in[LO_PC:, :]], outs=[tabs_hi[0][:]])

            def layer(l):
                tab_lo, tab_hi = tabs_lo[l], tabs_hi[l]
                # --- per-layer constants (double-buffered; prefetchable) ---
                wel = wts.tile([3, EMB], BF16, name="wel", tag="wel")
                nc.sync.dma_start(out=wel[:], in_=d["welb"][l])
                w1k = []
                for k in range(2):
                    t_ = wts.tile([128, HID], BF16, name=f"w1k{k}", tag=f"w1k{k}")
                    nc.sync.dma_start(out=t_[:], in_=d["w1"][l, k * 128:(k + 1) * 128, :])
                    w1k.append(t_)
                w2k = []
                for k in range(4):
                    t_ = wts.tile([128, EMB], BF16, name=f"w2k{k}", tag=f"w2k{k}")
                    nc.sync.dma_start(out=t_[:], in_=d["w2"][l, k * 128:(k + 1) * 128, :])
                    w2k.append(t_)
                gam, bet = [], []
                for m in range(4):
                    g_ = wts.tile([128, 1], F32, name=f"gam{m}", tag=f"gam{m}")
                    nc.sync.dma_start(out=g_[:], in_=d["gamma"][l, m * 128:(m + 1) * 128, :])
                    gam.append(g_)
                    b_ = wts.tile([128, 1], F32, name=f"bet{m}", tag=f"bet{m}")
                    nc.sync.dma_start(out=b_[:], in_=d["beta"][l, m * 128:(m + 1) * 128, :])
                    bet.append(b_)
                epsb = wts.tile([128, 1], F32, name="epsb", tag="epsb")
                nc.sync.dma_start(out=epsb[:], in_=d["epsr"][l])
                b2row = wts.tile([1, EMB], BF16, name="b2row", tag="b2row")
                nc.sync.dma_start(out=b2row[:], in_=d["b2b"][l])
                sacc, qacc = [], []
                for m in range(4):
                    s_ = spool.tile([128, 1], F32, name=f"sacc{m}", tag=f"sacc{m}")
                    nc.gpsimd.memset(s_[:], 0.0)
                    sacc.append(s_)
                    q_ = spool.tile([128, 1], F32, name=f"qacc{m}", tag=f"qacc{m}")
                    nc.gpsimd.memset(q_[:], 0.0)
                    qacc.append(q_)

                # ---- pass A: edges + MLP1 + stats ----
                for ch in chunks:
                    held = {}
                    for wi, w in enumerate(ch):
                        TL, TH = TLs[w], THs[w]
                        TLt = TL // 128
                        tpw = tpws[w]
                        eat = mpool.tile([3, TPWMAX * 128], BF16, name="eat", tag="eat")
                        nc.sync.dma_start(out=eat[:, :tpw * 128],
                                          in_=d["eattr"][w, :, 0:tpw * 128])
                        indt = mpool.tile([128, TPWMAX * 128], BF16, name="indt", tag="indt")
                        nc.sync.dma_start(out=indt[:, :tpw * 128],
                                          in_=d["indw"][w, :, 0:tpw * 128])
                        g = gbufs[w % 4]
                        if TL > 0:
                            ilo = mpool.tile([128, TLMAX // 16], I16, name="ilo", tag="ilo")
                            nc.sync.dma_start(out=ilo[:, :TL // 16],
                                              in_=d["idxlo"][w, :, 0:TL // 16])
                            nlo_reg = cnt_regs[(2 * w) % 4]
                            nc.gpsimd.reg_load(nlo_reg, cntt[0:1, w:w + 1])
                            nc.gpsimd.dma_gather(
                                out_ap=g[:, 0:TLt, :], in_ap=tab_lo[:],
                                idxs_ap=ilo[:, :TL // 16],
                                num_idxs=TL, num_idxs_reg=nlo_reg, elem_size=EMB,
                                queue_num=(2 * w) % 4)
                        held[w] = (eat, indt, g)
                    for wi, w in enumerate(ch):
                        TL, TH = TLs[w], THs[w]
                        TLt = TL // 128
                        tpw = tpws[w]
                        g = held[w][2]
                        if TH > 0:
                            ihi = mpool.tile([128, THMAX // 16], I16, name="ihi", tag="ihi")
                            nc.sync.dma_start(out=ihi[:, :TH // 16],
                                              in_=d["idxhi"][w, :, 0:TH // 16])
                            nhi_reg = cnt_regs[(2 * w + 1) % 4]
                            nc.gpsimd.reg_load(nhi_reg, cntt[0:1, WPC + w:WPC + w + 1])
                            nc.gpsimd.dma_gather(
                                out_ap=g[:, TLt:tpw, :], in_ap=tab_hi[:],
                                idxs_ap=ihi[:, :TH // 16],
                                num_idxs=TH, num_idxs_reg=nhi_reg, elem_size=EMB,
                                queue_num=(2 * w + 1) % 4)
                    for wi, w in enumerate(ch):
                        TL, TH = TLs[w], THs[w]
                        TLt = TL // 128
                        tpw = tpws[w]
                        eat, indt, g = held[w]
                        msg = wpool.tile([128, TPWMAX * EMB], BF16, name="msg", tag="msg")
                        for grp in range((tpw + 1) // 2):
                            t0 = grp * 2
                            ntile = min(2, tpw - t0)
                            cols = ntile * EMB
                            pe = psE.tile([128, 512], F32, name="pe", tag="pe")
                            nc.tensor.matmul(
                                out=pe[:, :cols], lhsT=identb[:],
                                rhs=g[:, t0:t0 + ntile, :],
                                start=True, stop=False)
                            for j in range(ntile):
                                nc.tensor.matmul(
                                    out=pe[:, j * EMB:(j + 1) * EMB],
                                    lhsT=eat[:, (t0 + j) * 128:(t0 + j + 1) * 128],
                                    rhs=wel[:], start=False, stop=(j == ntile - 1))
                            ms = msg[:, t0 * EMB:t0 * EMB + cols]
                            if (w + grp) % 2 == 0:
                                nc.scalar.activation(ms, pe[:, :cols], AF.Relu)
                            else:
                                nc.vector.tensor_scalar_max(ms, pe[:, :cols], 0.0)
                        agg = psA.tile([128, EMB], F32, name="agg", tag="agg")
                        for ti in range(tpw):
                            nc.tensor.matmul(
                                out=agg[:],
                                lhsT=indt[:, ti * 128:(ti + 1) * 128],
                                rhs=msg[:, ti * EMB:(ti + 1) * EMB],
                                start=(ti == 0), stop=(ti == tpw - 1))
                        z = wpool.tile([128, EMB], BF16, name="z", tag="z")
                        nc.vector.scalar_tensor_tensor(
                            out=z[:], in0=ownh[:, w * EMB:(w + 1) * EMB],
                            scalar=epsb[:, 0:1], in1=agg[:],
                            op0=OP.mult, op1=OP.add)
                        for k in range(2):
                            nc.sync.dma_start(
                                out=zT[k][:, wi * 128:(wi + 1) * 128],
                                in_=z[:, k * 128:(k + 1) * 128], transpose=True)
                    cs = len(ch) * 128
                    co = ch[0] * 128
                    for m in range(4):
                        py = psY.tile([128, 512], F32, name="py", tag="py")
                        for k in range(2):
                            nc.tensor.matmul(
                                out=py[:, :cs],
                                lhsT=w1k[k][:, m * 128:(m + 1) * 128],
                                rhs=zT[k][:, :cs], start=(k == 0), stop=(k == 1))
                        ys = y1t[m][:, co:co + cs]
                        t1 = wpool.tile([128, 1], F32, name="t1", tag="t1")
                        nc.scalar.activation(ys, py[:, :cs], AF.Copy, accum_out=t1[:])
                        nc.vector.tensor_add(out=sacc[m][:], in0=sacc[m][:], in1=t1[:])
                        t2 = wpool.tile([128, 1], F32, name="t2", tag="t2")
                        nc.vector.scalar_tensor_tensor(
                            out=scr[:, :cs], in0=ys, scalar=1.0,
                            in1=py[:, :cs], op0=OP.mult, op1=OP.mult,
                            accum_out=t2[:])
                        nc.vector.tensor_add(out=qacc[m][:], in0=qacc[m][:], in1=t2[:])

                # ---- stats AllReduce + scale/bias ----
                for m in range(4):
                    st = wpool.tile([128, 2], F32, name="st", tag="st")
                    nc.vector.tensor_copy(out=st[:, 0:1], in_=sacc[m][:])
                    nc.vector.tensor_copy(out=st[:, 1:2], in_=qacc[m][:])
                    nc.sync.dma_start(out=stats_in[m * 128:(m + 1) * 128, :], in_=st[:])
                stats_out = stats_outs[l]
                nc.gpsimd.collective_compute(
                    "AllReduce", OP.add, replica_groups=[list(range(NCORES))],
                    ins=[stats_in[:]], outs=[stats_out[:]])
                sca, tbi = [], []
                for m in range(4):
                    st2 = wpool.tile([128, 2], F32, name="st2", tag="st2")
                    nc.sync.dma_start(out=st2[:], in_=stats_out[m * 128:(m + 1) * 128, :])
                    mu = wpool.tile([128, 1], F32, name="mu", tag="mu")
                    nc.vector.tensor_scalar_mul(mu[:], st2[:, 0:1], 1.0 / N_NODES)
                    var = wpool.tile([128, 1], F32, name="var", tag="var")
                    nc.vector.tensor_scalar_mul(var[:], st2[:, 1:2], 1.0 / N_NODES)
                    msq = wpool.tile([128, 1], F32, name="msq", tag="msq")
                    nc.vector.tensor_tensor(out=msq[:], in0=mu[:], in1=mu[:], op=OP.mult)
                    nc.vector.tensor_tensor(out=var[:], in0=var[:], in1=msq[:], op=OP.subtract)
                    nc.vector.tensor_scalar_add(var[:], var[:], BN_EPS)
                    sd = wpool.tile([128, 1], F32, name="sd", tag="sd")
                    nc.scalar.activation(sd[:], var[:], AF.Sqrt)
                    istd = wpool.tile([128, 1], F32, name="istd", tag="istd")
                    nc.vector.reciprocal(istd[:], sd[:])
                    s_ = spool.tile([128, 1], F32, name=f"sc{m}", tag=f"sc{m}")
                    nc.vector.tensor_tensor(out=s_[:], in0=gam[m][:], in1=istd[:], op=OP.mult)
                    sca.append(s_)
                    tmp = wpool.tile([128, 1], F32, name="tmp", tag="tmp")
                    nc.vector.tensor_tensor(out=tmp[:], in0=mu[:], in1=s_[:], op=OP.mult)
                    tb_ = spool.tile([128, 1], F32, name=f"tb{m}", tag=f"tb{m}")
                    nc.vector.tensor_tensor(out=tb_[:], in0=bet[m][:], in1=tmp[:], op=OP.subtract)
                    tbi.append(tb_)

                # ---- pass B: BN-apply + MLP2 + table write ----
                for ci, ch in enumerate(chunks):
                    cs = len(ch) * 128
                    co = ch[0] * 128
                    for m in range(4):
                        nc.scalar.activation(y1t[m][:, co:co + cs],
                                             y1t[m][:, co:co + cs],
                                             AF.Relu, bias=tbi[m][:, 0:1],
                                             scale=sca[m][:, 0:1])
                    for w in ch:
                        ph = psH.tile([128, EMB], F32, name="ph", tag="ph")
                        nc.tensor.matmul(out=ph[:], lhsT=ones1[:],
                                         rhs=b2row[:], start=True, stop=False)
                        for k in range(4):
                            nc.tensor.matmul(out=ph[:],
                                             lhsT=y1t[k][:, w * 128:(w + 1) * 128],
                                             rhs=w2k[k][:], start=False, stop=(k == 3))
                        if l < L - 1:
                            hm = ownh[:, w * EMB:(w + 1) * EMB]
                            nc.scalar.activation(hm, ph[:], AF.Relu,
                                                 scale=maskw[:, w:w + 1])
                            nc.sync.dma_start(out=cc_in[w * 128:(w + 1) * 128, :], in_=hm)
                        else:
                            hn = wpool.tile([128, EMB], F32, name="hn", tag="hn")
                            nc.scalar.activation(hn[:], ph[:], AF.Copy)
                            nc.sync.dma_start(out=out_ext[w * 128:(w + 1) * 128, :], in_=hn[:])
                    if l < L - 1 and ci == ci_lo:
                        # windows 0..WLO-1 written: publish table half 1 early
                        nc.gpsimd.collective_compute(
                            "AllGather", OP.bypass,
                            replica_groups=[list(range(NCORES))],
                            ins=[cc_in[0:LO_PC, :]], outs=[tabs_lo[l + 1][:]])
                if l < L - 1:
                    nc.gpsimd.collective_compute(
                        "AllGather", OP.bypass,
                        replica_groups=[list(range(NCORES))],
                        ins=[cc_in[LO_PC:, :]], outs=[tabs_hi[l + 1][:]])

            prologue()
            for l in range(L):
                layer(l)

    nc.compile()
    _cache[key] = nc
    return nc


def _ceil128(x):
    return ((int(x) + 127) // 128) * 128


def _wrap16(flat, n_pad):
    """flat int array -> [128, n_pad//16] int16 (16-wrap, replicated 8x).

    Positions >= len(flat) are -1 (skipped by dma_gather)."""
    a = np.full(n_pad, -1, np.int16)
    a[:len(flat)] = flat.astype(np.int16)
    w = a.reshape(n_pad // 16, 16).T  # [16, n/16]
    return np.tile(w, (8, 1))


def _host_prep(node_ids, node_depth, edge_index, edge_attr, node_type_emb,
               depth_emb):
    """Bin-pack nodes, build per-core gather/indicator/attr arrays."""
    ids = np.asarray(node_ids).astype(np.int64).ravel()
    dep = np.clip(np.asarray(node_depth).astype(np.int64).ravel(), 0, 20)
    src = np.asarray(edge_index[0]).astype(np.int64).ravel()
    dst = np.asarray(edge_index[1]).astype(np.int64).ravel()
    attr = np.asarray(edge_attr, dtype=np.float32)

    deg = np.bincount(dst, minlength=N_NODES)
    order = np.argsort(-deg, kind="stable")
    bin_of = np.empty(N_NODES, np.int32)
    slot_of = np.empty(N_NODES, np.int32)
    counts = np.zeros(BINS, np.int32)
    loads = np.zeros(BINS, np.int64)
    fwd = np.arange(BINS)
    rev = fwd[::-1]
    pos = 0
    rnd = 0
    while pos < N_NODES:
        seq = fwd if rnd % 2 == 0 else rev
        for b in seq:
            if pos >= N_NODES:
                break
            if counts[b] >= 128:
                continue
            v = order[pos]
            bin_of[v] = b
            slot_of[v] = counts[b]
            counts[b] += 1
            loads[b] += deg[v]
            pos += 1
        rnd += 1
    target = int(np.ceil(loads.sum() / BINS / 128.0)) * 128
    members = [list(np.where(bin_of == b)[0]) for b in range(BINS)]
    for _ in range(2000):
        a = int(loads.argmax())
        if loads[a] <= target:
            break
        b = int(loads.argmin())
        na = max(members[a], key=lambda v: deg[v])
        nb = min(members[b], key=lambda v: deg[v])
        if deg[na] <= deg[nb]:
            break
        members[a].remove(na); members[b].remove(nb)
        members[a].append(nb); members[b].append(na)
        loads[a] += deg[nb] - deg[na]
        loads[b] += deg[na] - deg[nb]
    for b in range(BINS):
        for s, v in enumerate(members[b]):
            bin_of[v] = b
            slot_of[v] = s

    # table rows: [cores x windows 0..WLO-1 | cores x windows WLO..]
    c_all, w_all = np.divmod(bin_of, WPC)
    row_tab = np.where(
        w_all < WLO,
        c_all * LO_PC + w_all * 128 + slot_of,
        ROWS_LO + c_all * HI_PC + (w_all - WLO) * 128 + slot_of).astype(np.int64)
    row_out = bin_of.astype(np.int64) * 128 + slot_of

    srcrow = row_tab[src]
    dslot = slot_of[dst]
    ebin = bin_of[dst]
    eorder = np.argsort(ebin, kind="stable")
    bounds = np.searchsorted(ebin[eorder], np.arange(BINS + 1))
    lo_lists = [[[] for _ in range(WPC)] for _ in range(NCORES)]
    hi_lists = [[[] for _ in range(WPC)] for _ in range(NCORES)]
    nlo = np.zeros((NCORES, WPC), np.int64)
    nhi = np.zeros((NCORES, WPC), np.int64)
    for b in range(BINS):
        c, w = divmod(b, WPC)
        el = eorder[bounds[b]:bounds[b + 1]]
        el = el[np.argsort(srcrow[el], kind="stable")]
        is_lo = srcrow[el] < ROWS_LO
        lo_lists[c][w] = el[is_lo]
        hi_lists[c][w] = el[~is_lo]
        nlo[c, w] = is_lo.sum()
        nhi[c, w] = len(el) - is_lo.sum()
    TLs = tuple(_ceil128(max(nlo[:, w].max(), 1)) for w in range(WPC))
    THs = tuple(_ceil128(max(nhi[:, w].max(), 1)) for w in range(WPC))
    TLMAX, THMAX = max(TLs), max(THs)
    tpws = [(TLs[w] + THs[w]) // 128 for w in range(WPC)]
    TPWMAX = max(tpws)

    idxlo = np.zeros((NCORES, WPC, 128, TLMAX // 16), np.int16)
    idxhi = np.zeros((NCORES, WPC, 128, THMAX // 16), np.int16)
    cnts = np.zeros((NCORES, 1, 2 * WPC), np.int32)
    eattr = np.zeros((NCORES, WPC, 3, TPWMAX * 128), np.float32)
    indw = np.zeros((NCORES, WPC, 128, TPWMAX * 128), np.float32)
    for c in range(NCORES):
        for w in range(WPC):
            ello, elhi = lo_lists[c][w], hi_lists[c][w]
            lo_rows = srcrow[ello]
            hi_rows = srcrow[elhi] - ROWS_LO
            if len(lo_rows) == 0:
                lo_rows = np.array([0], np.int64)  # 1 junk row; ind col 0
            if len(hi_rows) == 0:
                hi_rows = np.array([0], np.int64)
            idxlo[c, w, :, :TLs[w] // 16] = _wrap16(lo_rows, TLs[w])
            idxhi[c, w, :, :THs[w] // 16] = _wrap16(hi_rows, THs[w])
            cnts[c, 0, w] = len(lo_rows)
            cnts[c, 0, WPC + w] = len(hi_rows)
            for el, base in ((ello, 0), (elhi, TLs[w])):
                n = len(el)
                if n == 0:
                    continue
                flat = base + np.arange(n)
                ti, p = np.divmod(flat, 128)
                eattr[c, w, 0, flat] = attr[el, 0]
                eattr[c, w, 1, flat] = attr[el, 1]
                eattr[c, w, 2, flat] = 1.0
                indw[c, w, p, ti * 128 + dslot[el]] = 1.0

    maskpw = np.zeros((NCORES, 128, WPC), np.float32)
    maskpw[c_all, slot_of, w_all] = 1.0

    h0 = (np.asarray(node_type_emb, np.float32)[ids]
          + np.asarray(depth_emb, np.float32)[dep])  # [N, EMB]
    h0f = np.zeros((NCORES, WPC, 128, EMB), np.float32)
    h0f[c_all, w_all, slot_of] = h0

    return (TLs, THs), row_out, dict(
        idxlo=idxlo, idxhi=idxhi, cnts=cnts,
        eattr=eattr.astype(BF), indw=indw.astype(BF),
        maskpw=maskpw, h0b=h0f.astype(BF))


def _prepare(node_ids, node_depth, edge_index, edge_attr, node_type_emb,
             depth_emb, We, be, W1, b1, gamma, beta, W2, b2, eps_param):
    key, row_out, per = _host_prep(node_ids, node_depth, edge_index, edge_attr,
                                   node_type_emb, depth_emb)
    nc = _build(key)

    we_aug = np.concatenate([np.asarray(We, np.float32),
                             np.asarray(be, np.float32)[:, None, :]], axis=1)
    common = {
        "welb": we_aug.astype(BF),
        "w1": np.asarray(W1, np.float32).astype(BF),
        "w2": np.asarray(W2, np.float32).astype(BF),
        "gamma": np.asarray(gamma, np.float32).reshape(L, HID, 1),
        "beta": np.asarray(beta, np.float32).reshape(L, HID, 1),
        "b2b": np.asarray(b2, np.float32).reshape(L, 1, EMB).astype(BF),
        "epsr": np.broadcast_to(
            (1.0 + np.asarray(eps_param, np.float32))[:, None, None],
            (L, 128, 1)).copy(),
    }
    in_maps = []
    for c in range(NCORES):
        m = dict(common)
        for k in ("idxlo", "idxhi", "cnts", "eattr", "indw", "maskpw", "h0b"):
            m[k] = per[k][c]
        in_maps.append(m)
    return nc, in_maps, row_out


def _assemble(res, row_out):
    full = np.concatenate([res.results[c]["out"] for c in range(NCORES)], axis=0)
    return full[row_out].astype(np.float32)


def kernel(**inputs):
    nc, in_maps, row_out = _prepare(**inputs)
    res = bass_utils.run_bass_kernel_spmd(nc, in_maps, core_ids=list(range(NCORES)))
    return _assemble(res, row_out)


# revision 15
# speedup vs baseline: 1.2525x; 1.2525x over previous
"""GNN message-passing kernel for Trainium2, 8 NeuronCores.

Sharding: destination-node partition; 8 cores x 49 windows of <=128 nodes.
Host precomputes node embeddings (h0), per-window edge slot layouts (edges
split by source table half for int16 dma_gather indices, ragged per-window
tile counts), edge-attr columns and 0/1 indicator tiles (bf16).

Table rows are laid out as [all cores' windows 0..WLO-1 | rest] so the
next-layer table is published with TWO AllGathers; lo-half gathers of the
next layer lead hi-half gathers by two chunks so the hi AllGather hides
behind the Q7 descriptor-generation stream. Gather num_idxs is the exact
max-over-cores count per (window, half) (Q7 desc-gen cost is ~8ns per
static index, so padding indexes cost real time).

Per layer on each core: e = eat @ [We;be] on PE (bf16), h-add via identity
matmul into PSUM, msg = relu(.) (Scalar/Vector), segment-sum via indicator
matmul, own-h SBUF-resident (bf16), MLP1 (bf16) via PE-transposed zT, BN
batch stats via accum_out + AllReduce, BN-apply+ReLU on ScalarE, MLP2
(bf16) with b2 folded into PSUM via a K=1 ones-row matmul, table published
via split bf16 AllGather.
"""
import sys
import numpy as np
import ml_dtypes

sys.path.insert(0, "/opt/trn_rl_repo")

import concourse.bass as bass
import concourse.bacc as bacc
import concourse.mybir as mybir
import concourse.tile as tile
from concourse import bass_utils
from concourse.masks import make_identity

F32 = mybir.dt.float32
BF16 = mybir.dt.bfloat16
I32 = mybir.dt.int32
I16 = mybir.dt.int16
OP = mybir.AluOpType
AF = mybir.ActivationFunctionType
BF = ml_dtypes.bfloat16

N_NODES = 50000
N_EDGES = 300000
EMB = 256
HID = 512
L = 5
NCORES = 8
WPC = 49                      # windows per core
BINS = NCORES * WPC           # 392
ROWS_PC = WPC * 128           # 6272 padded rows per core
WLO = 22                      # windows in the first table half
LO_PC = WLO * 128             # rows per core in half 1
HI_PC = (WPC - WLO) * 128     # rows per core in half 2
ROWS_LO = NCORES * LO_PC      # int16-safe gather range
BN_EPS = 1e-5
LEAD = 2                      # lo-gather chunk lead over hi-gather/compute

_cache = {}


def _ceil(x, m):
    return ((int(x) + m - 1) // m) * m


def _build(key):
    """Build + compile the SPMD program.

    key = (TLs, THs, NLOs, NHIs): per-window 128-padded slot layout bounds
    and exact static gather counts (max over cores)."""
    if key in _cache:
        return _cache[key]
    TLs, THs, NLOs, NHIs = key
    TLMAX, THMAX = max(TLs), max(THs)
    tpws = [(TLs[w] + THs[w]) // 128 for w in range(WPC)]
    TPWMAX = max(tpws)
    TLTMAX = max(TLs) // 128
    THTMAX = max(THs) // 128
    nc = bacc.Bacc("TRN2", target_bir_lowering=False, debug=False,
                   num_devices=NCORES, num_swdge_queues=4)
    d = {}
    d["h0b"] = nc.dram_tensor("h0b", [WPC, 128, EMB], BF16, kind="ExternalInput").ap()
    d["idxlo"] = nc.dram_tensor("idxlo", [WPC, 128, TLMAX // 16], I16, kind="ExternalInput").ap()
    d["idxhi"] = nc.dram_tensor("idxhi", [WPC, 128, THMAX // 16], I16, kind="ExternalInput").ap()
    d["eattr"] = nc.dram_tensor("eattr", [WPC, 3, TPWMAX * 128], BF16, kind="ExternalInput").ap()
    d["indw"] = nc.dram_tensor("indw", [WPC, 128, TPWMAX * 128], BF16, kind="ExternalInput").ap()
    d["maskpw"] = nc.dram_tensor("maskpw", [128, WPC], F32, kind="ExternalInput").ap()
    d["welb"] = nc.dram_tensor("welb", [L, 3, EMB], BF16, kind="ExternalInput").ap()
    d["w1"] = nc.dram_tensor("w1", [L, EMB, HID], BF16, kind="ExternalInput").ap()
    d["w2"] = nc.dram_tensor("w2", [L, HID, EMB], BF16, kind="ExternalInput").ap()
    d["gamma"] = nc.dram_tensor("gamma", [L, HID, 1], F32, kind="ExternalInput").ap()
    d["beta"] = nc.dram_tensor("beta", [L, HID, 1], F32, kind="ExternalInput").ap()
    d["b2b"] = nc.dram_tensor("b2b", [L, 1, EMB], BF16, kind="ExternalInput").ap()
    d["epsr"] = nc.dram_tensor("epsr", [L, 128, 1], F32, kind="ExternalInput").ap()
    out_ext = nc.dram_tensor("out", [ROWS_PC, EMB], F32, kind="ExternalOutput").ap()

    chunks = [list(range(c * 4, min(c * 4 + 4, WPC))) for c in range((WPC + 3) // 4)]
    nch = len(chunks)
    # first chunk index by which windows 0..WLO-1 are all written
    ci_lo = min(ci for ci, ch in enumerate(chunks) if ch[-1] >= WLO - 1)
    NGLO = 4 * (LEAD + 1)     # live lo-gather window buffers
    NGHI = 6                  # live hi-gather window buffers

    with tile.TileContext(nc) as tc:
        with tc.tile_pool(name="const", bufs=1) as cpool, \
             tc.tile_pool(name="wts", bufs=2) as wts, \
             tc.tile_pool(name="meta", bufs=6) as mpool, \
             tc.tile_pool(name="work", bufs=4) as wpool, \
             tc.tile_pool(name="resid", bufs=1) as rpool, \
             tc.tile_pool(name="stat", bufs=1) as spool, \
             tc.tile_pool(name="psA", bufs=2, space="PSUM") as psA, \
             tc.tile_pool(name="psE", bufs=2, space="PSUM") as psE, \
             tc.tile_pool(name="psT", bufs=1, space="PSUM") as psT, \
             tc.tile_pool(name="psY", bufs=2, space="PSUM") as psY, \
             tc.tile_pool(name="psH", bufs=1, space="PSUM") as psH, \
             tc.tile_pool(name="dram", bufs=1, space="DRAM") as dpool:

            # ---- constants ----
            ident = cpool.tile([128, 128], F32, name="ident")
            make_identity(nc, ident[:])
            identb = cpool.tile([128, 128], BF16, name="identb")
            nc.vector.tensor_copy(out=identb[:], in_=ident[:])
            maskw = cpool.tile([128, WPC], F32, name="maskw")
            nc.sync.dma_start(out=maskw[:], in_=d["maskpw"][:])
            ones1 = cpool.tile([1, 128], BF16, name="ones1")
            nc.gpsimd.memset(ones1[:], 1.0)

            # ---- DRAM scratch ----
            cc_in = dpool.tile([ROWS_PC, EMB], BF16, name="cc_in")
            tabs_lo = [dpool.tile([ROWS_LO, EMB], BF16, name=f"tabl{l}",
                                  addr_space="Shared") for l in range(L)]
            tabs_hi = [dpool.tile([NCORES * ROWS_PC - ROWS_LO, EMB], BF16,
                                  name=f"tabh{l}",
                                  addr_space="Shared") for l in range(L)]
            stats_in = dpool.tile([HID, 2], F32, name="stats_in")
            stats_outs = [dpool.tile([HID, 2], F32, name=f"stats_out{l}",
                                     addr_space="Shared") for l in range(L)]

            # ---- residents ----
            y1t = [rpool.tile([128, ROWS_PC], BF16, name=f"y1t{m}") for m in range(4)]
            zT = [rpool.tile([128, 512], BF16, name=f"zT{k}", bufs=2) for k in range(2)]
            scr = rpool.tile([128, 512], BF16, name="scr")
            ownh = rpool.tile([128, WPC * EMB], BF16, name="ownh")
            glo = [rpool.tile([128, TLTMAX, EMB], BF16, name=f"glo{i}")
                   for i in range(NGLO)]
            ghi = [rpool.tile([128, THTMAX, EMB], BF16, name=f"ghi{i}")
                   for i in range(NGHI)]

            def prologue():
                # own-h resident from host h0 (bf16)
                for w in range(WPC):
                    nc.sync.dma_start(out=ownh[:, w * EMB:(w + 1) * EMB],
                                      in_=d["h0b"][w])
                # seed cc_in lo-half first so its AllGather starts early
                def seed(w0, w1):
                    for w in range(w0, w1, 7):
                        hb = wpool.tile([128, 7 * EMB], BF16, name="hb", tag="hb")
                        nw = min(7, w1 - w)
                        rows = nw * 128
                        nc.sync.dma_start(
                            out=hb[:, :rows * EMB // 128],
                            in_=d["h0b"][w:w + nw])
                        nc.sync.dma_start(
                            out=cc_in[w * 128:w * 128 + rows, :],
                            in_=hb[:, :rows * EMB // 128])
                seed(0, WLO)
                nc.gpsimd.collective_compute(
                    "AllGather", OP.bypass,
                    replica_groups=[list(range(NCORES))],
                    ins=[cc_in[0:LO_PC, :]], outs=[tabs_lo[0][:]])
                seed(WLO, WPC)
                # scrub gather buffers (junk slots must be finite: 0*NaN=NaN)
                for gb in glo + ghi:
                    nc.gpsimd.memset(gb[:], 0.0)
                nc.gpsimd.collective_compute(
                    "AllGather", OP.bypass,
                    replica_groups=[list(range(NCORES))],
                    ins=[cc_in[LO_PC:, :]], outs=[tabs_hi[0][:]])

            def gather_lo(w, tab_lo):
                TL = TLs[w]
                TLt = TL // 128
                nlo = NLOs[w]
                ilo = mpool.tile([128, TLMAX // 16], I16, name="ilo", tag="ilo")
                ncols = _ceil(nlo, 16) // 16
                nc.sync.dma_start(out=ilo[:, :ncols],
                                  in_=d["idxlo"][w, :, 0:ncols])
                g = glo[w % NGLO]
                nc.gpsimd.dma_gather(
                    out_ap=g[:, 0:TLt, :], in_ap=tab_lo[:],
                    idxs_ap=ilo[:, :ncols],
                    num_idxs=nlo, num_idxs_reg=nlo, elem_size=EMB,
                    queue_num=(2 * w) % 4)

            def gather_hi(w, tab_hi):
                TH = THs[w]
                THt = TH // 128
                nhi = NHIs[w]
                ihi = mpool.tile([128, THMAX // 16], I16, name="ihi", tag="ihi")
                ncols = _ceil(nhi, 16) // 16
                nc.sync.dma_start(out=ihi[:, :ncols],
                                  in_=d["idxhi"][w, :, 0:ncols])
                g = ghi[w % NGHI]
                nc.gpsimd.dma_gather(
                    out_ap=g[:, 0:THt, :], in_ap=tab_hi[:],
                    idxs_ap=ihi[:, :ncols],
                    num_idxs=nhi, num_idxs_reg=nhi, elem_size=EMB,
                    queue_num=(2 * w + 1) % 4)

            def layer(l):
                tab_lo, tab_hi = tabs_lo[l], tabs_hi[l]
                # --- per-layer constants (double-buffered; prefetchable) ---
                wel = wts.tile([3, EMB], BF16, name="wel", tag="wel")
                nc.sync.dma_start(out=wel[:], in_=d["welb"][l])
                w1k = []
                for k in range(2):
                    t_ = wts.tile([128, HID], BF16, name=f"w1k{k}", tag=f"w1k{k}")
                    nc.sync.dma_start(out=t_[:], in_=d["w1"][l, k * 128:(k + 1) * 128, :])
                    w1k.append(t_)
                w2k = []
                for k in range(4):
                    t_ = wts.tile([128, EMB], BF16, name=f"w2k{k}", tag=f"w2k{k}")
                    nc.sync.dma_start(out=t_[:], in_=d["w2"][l, k * 128:(k + 1) * 128, :])
                    w2k.append(t_)
                gam, bet = [], []
                for m in range(4):
                    g_ = wts.tile([128, 1], F32, name=f"gam{m}", tag=f"gam{m}")
                    nc.sync.dma_start(out=g_[:], in_=d["gamma"][l, m * 128:(m + 1) * 128, :])
                    gam.append(g_)
                    b_ = wts.tile([128, 1], F32, name=f"bet{m}", tag=f"bet{m}")
                    nc.sync.dma_start(out=b_[:], in_=d["beta"][l, m * 128:(m + 1) * 128, :])
                    bet.append(b_)
                epsb = wts.tile([128, 1], F32, name="epsb", tag="epsb")
                nc.sync.dma_start(out=epsb[:], in_=d["epsr"][l])
                b2row = wts.tile([1, EMB], BF16, name="b2row", tag="b2row")
                nc.sync.dma_start(out=b2row[:], in_=d["b2b"][l])
                sacc, qacc = [], []
                for m in range(4):
                    s_ = spool.tile([128, 1], F32, name=f"sacc{m}", tag=f"sacc{m}")
                    nc.gpsimd.memset(s_[:], 0.0)
                    sacc.append(s_)
                    q_ = spool.tile([128, 1], F32, name=f"qacc{m}", tag=f"qacc{m}")
                    nc.gpsimd.memset(q_[:], 0.0)
                    qacc.append(q_)

                # ---- pass A: edges + MLP1 + stats ----
                # lo gathers lead by LEAD chunks so the hi AllGather hides
                # behind the Q7 lo-gather stream.
                for ci in range(min(LEAD, nch)):
                    for w in chunks[ci]:
                        gather_lo(w, tab_lo)
                for ci, ch in enumerate(chunks):
                    if ci + LEAD < nch:
                        for w in chunks[ci + LEAD]:
                            gather_lo(w, tab_lo)
                    for w in ch:
                        gather_hi(w, tab_hi)
                    held = {}
                    for w in ch:
                        tpw = tpws[w]
                        eat = mpool.tile([3, TPWMAX * 128], BF16, name="eat", tag="eat")
                        nc.sync.dma_start(out=eat[:, :tpw * 128],
                                          in_=d["eattr"][w, :, 0:tpw * 128])
                        indt = mpool.tile([128, TPWMAX * 128], BF16, name="indt", tag="indt")
                        nc.sync.dma_start(out=indt[:, :tpw * 128],
                                          in_=d["indw"][w, :, 0:tpw * 128])
                        held[w] = (eat, indt)
                    for wi, w in enumerate(ch):
                        TLt = TLs[w] // 128
                        THt = THs[w] // 128
                        tpw = tpws[w]
                        eat, indt = held[w]
                        gl, gh = glo[w % NGLO], ghi[w % NGHI]
                        msg = wpool.tile([128, TPWMAX * EMB], BF16, name="msg", tag="msg")
                        # groups of <=2 tiles within each half
                        groups = []
                        for t0 in range(0, TLt, 2):
                            groups.append((gl, t0, min(2, TLt - t0), t0))
                        for t0 in range(0, THt, 2):
                            groups.append((gh, t0, min(2, THt - t0), TLt + t0))
                        for gi_, (gsrc, t0, ntile, mt0) in enumerate(groups):
                            cols = ntile * EMB
                            pe = psE.tile([128, 512], F32, name="pe", tag="pe")
                            nc.tensor.matmul(
                                out=pe[:, :cols], lhsT=identb[:],
                                rhs=gsrc[:, t0:t0 + ntile, :],
                                start=True, stop=False)
                            for j in range(ntile):
                                nc.tensor.matmul(
                                    out=pe[:, j * EMB:(j + 1) * EMB],
                                    lhsT=eat[:, (mt0 + j) * 128:(mt0 + j + 1) * 128],
                                    rhs=wel[:], start=False, stop=(j == ntile - 1))
                            ms = msg[:, mt0 * EMB:mt0 * EMB + cols]
                            if (w + gi_) % 4 == 0:
                                nc.scalar.activation(ms, pe[:, :cols], AF.Relu)
                            else:
                                nc.vector.tensor_scalar_max(ms, pe[:, :cols], 0.0)
                        agg = psA.tile([128, EMB], F32, name="agg", tag="agg")
                        for ti in range(tpw):
                            nc.tensor.matmul(
                                out=agg[:],
                                lhsT=indt[:, ti * 128:(ti + 1) * 128],
                                rhs=msg[:, ti * EMB:(ti + 1) * EMB],
                                start=(ti == 0), stop=(ti == tpw - 1))
                        z = wpool.tile([128, EMB], BF16, name="z", tag="z")
                        nc.vector.scalar_tensor_tensor(
                            out=z[:], in0=ownh[:, w * EMB:(w + 1) * EMB],
                            scalar=epsb[:, 0:1], in1=agg[:],
                            op0=OP.mult, op1=OP.add)
                        for k in range(2):
                            tp = psT.tile([128, 128], BF16, name="tp", tag="tp")
                            nc.tensor.transpose(out=tp[:], in_=z[:, k * 128:(k + 1) * 128],
                                                identity=identb[:])
                            nc.vector.tensor_copy(out=zT[k][:, wi * 128:(wi + 1) * 128],
                                                  in_=tp[:])
                    cs = len(ch) * 128
                    co = ch[0] * 128
                    for m in range(4):
                        py = psY.tile([128, 512], F32, name="py", tag="py")
                        for k in range(2):
                            nc.tensor.matmul(
                                out=py[:, :cs],
                                lhsT=w1k[k][:, m * 128:(m + 1) * 128],
                                rhs=zT[k][:, :cs], start=(k == 0), stop=(k == 1))
                        ys = y1t[m][:, co:co + cs]
                        t1 = wpool.tile([128, 1], F32, name="t1", tag="t1")
                        nc.scalar.activation(ys, py[:, :cs], AF.Copy, accum_out=t1[:])
                        nc.vector.tensor_add(out=sacc[m][:], in0=sacc[m][:], in1=t1[:])
                        t2 = wpool.tile([128, 1], F32, name="t2", tag="t2")
                        nc.vector.scalar_tensor_tensor(
                            out=scr[:, :cs], in0=ys, scalar=1.0,
                            in1=py[:, :cs], op0=OP.mult, op1=OP.mult,
                            accum_out=t2[:])
                        nc.vector.tensor_add(out=qacc[m][:], in0=qacc[m][:], in1=t2[:])

                # ---- stats AllReduce + scale/bias ----
                for m in range(4):
                    st = wpool.tile([128, 2], F32, name="st", tag="st")
                    nc.vector.tensor_copy(out=st[:, 0:1], in_=sacc[m][:])
                    nc.vector.tensor_copy(out=st[:, 1:2], in_=qacc[m][:])
                    nc.sync.dma_start(out=stats_in[m * 128:(m + 1) * 128, :], in_=st[:])
                stats_out = stats_outs[l]
                nc.gpsimd.collective_compute(
                    "AllReduce", OP.add, replica_groups=[list(range(NCORES))],
                    ins=[stats_in[:]], outs=[stats_out[:]])
                sca, tbi = [], []
                for m in range(4):
                    st2 = wpool.tile([128, 2], F32, name="st2", tag="st2")
                    nc.sync.dma_start(out=st2[:], in_=stats_out[m * 128:(m + 1) * 128, :])
                    mu = wpool.tile([128, 1], F32, name="mu", tag="mu")
                    nc.vector.tensor_scalar_mul(mu[:], st2[:, 0:1], 1.0 / N_NODES)
                    var = wpool.tile([128, 1], F32, name="var", tag="var")
                    nc.vector.tensor_scalar_mul(var[:], st2[:, 1:2], 1.0 / N_NODES)
                    msq = wpool.tile([128, 1], F32, name="msq", tag="msq")
                    nc.vector.tensor_tensor(out=msq[:], in0=mu[:], in1=mu[:], op=OP.mult)
                    nc.vector.tensor_tensor(out=var[:], in0=var[:], in1=msq[:], op=OP.subtract)
                    nc.vector.tensor_scalar_add(var[:], var[:], BN_EPS)
                    sd = wpool.tile([128, 1], F32, name="sd", tag="sd")
                    nc.scalar.activation(sd[:], var[:], AF.Sqrt)
                    istd = wpool.tile([128, 1], F32, name="istd", tag="istd")
                    nc.vector.reciprocal(istd[:], sd[:])
                    s_ = spool.tile([128, 1], F32, name=f"sc{m}", tag=f"sc{m}")
                    nc.vector.tensor_tensor(out=s_[:], in0=gam[m][:], in1=istd[:], op=OP.mult)
                    sca.append(s_)
                    tmp = wpool.tile([128, 1], F32, name="tmp", tag="tmp")
                    nc.vector.tensor_tensor(out=tmp[:], in0=mu[:], in1=s_[:], op=OP.mult)
                    tb_ = spool.tile([128, 1], F32, name=f"tb{m}", tag=f"tb{m}")
                    nc.vector.tensor_tensor(out=tb_[:], in0=bet[m][:], in1=tmp[:], op=OP.subtract)
                    tbi.append(tb_)

                # ---- pass B: BN-apply + MLP2 + table write ----
                for ci, ch in enumerate(chunks):
                    cs = len(ch) * 128
                    co = ch[0] * 128
                    for m in range(4):
                        nc.scalar.activation(y1t[m][:, co:co + cs],
                                             y1t[m][:, co:co + cs],
                                             AF.Relu, bias=tbi[m][:, 0:1],
                                             scale=sca[m][:, 0:1])
                    for w in ch:
                        ph = psH.tile([128, EMB], F32, name="ph", tag="ph")
                        nc.tensor.matmul(out=ph[:], lhsT=ones1[:],
                                         rhs=b2row[:], start=True, stop=False)
                        for k in range(4):
                            nc.tensor.matmul(out=ph[:],
                                             lhsT=y1t[k][:, w * 128:(w + 1) * 128],
                                             rhs=w2k[k][:], start=False, stop=(k == 3))
                        if l < L - 1:
                            hm = ownh[:, w * EMB:(w + 1) * EMB]
                            nc.scalar.activation(hm, ph[:], AF.Relu,
                                                 scale=maskw[:, w:w + 1])
                            nc.sync.dma_start(out=cc_in[w * 128:(w + 1) * 128, :], in_=hm)
                        else:
                            hn = wpool.tile([128, EMB], F32, name="hn", tag="hn")
                            nc.scalar.activation(hn[:], ph[:], AF.Copy)
                            nc.sync.dma_start(out=out_ext[w * 128:(w + 1) * 128, :], in_=hn[:])
                    if l < L - 1 and ci == ci_lo:
                        # windows 0..WLO-1 written: publish table half 1 early
                        nc.gpsimd.collective_compute(
                            "AllGather", OP.bypass,
                            replica_groups=[list(range(NCORES))],
                            ins=[cc_in[0:LO_PC, :]], outs=[tabs_lo[l + 1][:]])
                if l < L - 1:
                    nc.gpsimd.collective_compute(
                        "AllGather", OP.bypass,
                        replica_groups=[list(range(NCORES))],
                        ins=[cc_in[LO_PC:, :]], outs=[tabs_hi[l + 1][:]])

            prologue()
            for l in range(L):
                layer(l)

    nc.compile()
    _cache[key] = nc
    return nc


def _wrap16(flat, n_pad):
    """flat int array -> [128, n_pad//16] int16 (16-wrap, replicated 8x).

    Positions >= len(flat) are 0 (gathers junk row 0; indicator zeroes it)."""
    a = np.zeros(n_pad, np.int16)
    a[:len(flat)] = flat.astype(np.int16)
    w = a.reshape(n_pad // 16, 16).T  # [16, n/16]
    return np.tile(w, (8, 1))


def _host_prep(node_ids, node_depth, edge_index, edge_attr, node_type_emb,
               depth_emb):
    """Bin-pack nodes, build per-core gather/indicator/attr arrays."""
    ids = np.asarray(node_ids).astype(np.int64).ravel()
    dep = np.clip(np.asarray(node_depth).astype(np.int64).ravel(), 0, 20)
    src = np.asarray(edge_index[0]).astype(np.int64).ravel()
    dst = np.asarray(edge_index[1]).astype(np.int64).ravel()
    attr = np.asarray(edge_attr, dtype=np.float32)

    deg = np.bincount(dst, minlength=N_NODES)
    order = np.argsort(-deg, kind="stable")
    bin_of = np.empty(N_NODES, np.int32)
    slot_of = np.empty(N_NODES, np.int32)
    counts = np.zeros(BINS, np.int32)
    loads = np.zeros(BINS, np.int64)
    fwd = np.arange(BINS)
    rev = fwd[::-1]
    pos = 0
    rnd = 0
    while pos < N_NODES:
        seq = fwd if rnd % 2 == 0 else rev
        for b in seq:
            if pos >= N_NODES:
                break
            if counts[b] >= 128:
                continue
            v = order[pos]
            bin_of[v] = b
            slot_of[v] = counts[b]
            counts[b] += 1
            loads[b] += deg[v]
            pos += 1
        rnd += 1
    target = int(np.ceil(loads.sum() / BINS / 128.0)) * 128
    members = [list(np.where(bin_of == b)[0]) for b in range(BINS)]
    for _ in range(2000):
        a = int(loads.argmax())
        if loads[a] <= target:
            break
        b = int(loads.argmin())
        na = max(members[a], key=lambda v: deg[v])
        nb = min(members[b], key=lambda v: deg[v])
        if deg[na] <= deg[nb]:
            break
        members[a].remove(na); members[b].remove(nb)
        members[a].append(nb); members[b].append(na)
        loads[a] += deg[nb] - deg[na]
        loads[b] += deg[na] - deg[nb]
    for b in range(BINS):
        for s, v in enumerate(members[b]):
            bin_of[v] = b
            slot_of[v] = s

    # table rows: [cores x windows 0..WLO-1 | cores x windows WLO..]
    c_all, w_all = np.divmod(bin_of, WPC)
    row_tab = np.where(
        w_all < WLO,
        c_all * LO_PC + w_all * 128 + slot_of,
        ROWS_LO + c_all * HI_PC + (w_all - WLO) * 128 + slot_of).astype(np.int64)
    row_out = bin_of.astype(np.int64) * 128 + slot_of

    srcrow = row_tab[src]
    dslot = slot_of[dst]
    ebin = bin_of[dst]
    eorder = np.argsort(ebin, kind="stable")
    bounds = np.searchsorted(ebin[eorder], np.arange(BINS + 1))
    lo_lists = [[[] for _ in range(WPC)] for _ in range(NCORES)]
    hi_lists = [[[] for _ in range(WPC)] for _ in range(NCORES)]
    nlo = np.zeros((NCORES, WPC), np.int64)
    nhi = np.zeros((NCORES, WPC), np.int64)
    for b in range(BINS):
        c, w = divmod(b, WPC)
        el = eorder[bounds[b]:bounds[b + 1]]
        el = el[np.argsort(srcrow[el], kind="stable")]
        is_lo = srcrow[el] < ROWS_LO
        lo_lists[c][w] = el[is_lo]
        hi_lists[c][w] = el[~is_lo]
        nlo[c, w] = is_lo.sum()
        nhi[c, w] = len(el) - is_lo.sum()
    NLOs = tuple(_ceil(max(nlo[:, w].max(), 1), 16) for w in range(WPC))
    NHIs = tuple(_ceil(max(nhi[:, w].max(), 1), 16) for w in range(WPC))
    TLs = tuple(_ceil(NLOs[w], 128) for w in range(WPC))
    THs = tuple(_ceil(NHIs[w], 128) for w in range(WPC))
    TLMAX, THMAX = max(TLs), max(THs)
    tpws = [(TLs[w] + THs[w]) // 128 for w in range(WPC)]
    TPWMAX = max(tpws)

    idxlo = np.zeros((NCORES, WPC, 128, TLMAX // 16), np.int16)
    idxhi = np.zeros((NCORES, WPC, 128, THMAX // 16), np.int16)
    eattr = np.zeros((NCORES, WPC, 3, TPWMAX * 128), np.float32)
    indw = np.zeros((NCORES, WPC, 128, TPWMAX * 128), np.float32)
    for c in range(NCORES):
        for w in range(WPC):
            ello, elhi = lo_lists[c][w], hi_lists[c][w]
            lo_rows = srcrow[ello]
            hi_rows = srcrow[elhi] - ROWS_LO
            nl16 = _ceil(NLOs[w], 16)
            nh16 = _ceil(NHIs[w], 16)
            idxlo[c, w, :, :nl16 // 16] = _wrap16(lo_rows, nl16)
            idxhi[c, w, :, :nh16 // 16] = _wrap16(hi_rows, nh16)
            for el, base in ((ello, 0), (elhi, TLs[w])):
                n = len(el)
                if n == 0:
                    continue
                flat = base + np.arange(n)
                ti, p = np.divmod(flat, 128)
                eattr[c, w, 0, flat] = attr[el, 0]
                eattr[c, w, 1, flat] = attr[el, 1]
                eattr[c, w, 2, flat] = 1.0
                indw[c, w, p, ti * 128 + dslot[el]] = 1.0

    maskpw = np.zeros((NCORES, 128, WPC), np.float32)
    maskpw[c_all, slot_of, w_all] = 1.0

    h0 = (np.asarray(node_type_emb, np.float32)[ids]
          + np.asarray(depth_emb, np.float32)[dep])  # [N, EMB]
    h0f = np.zeros((NCORES, WPC, 128, EMB), np.float32)
    h0f[c_all, w_all, slot_of] = h0

    return (TLs, THs, NLOs, NHIs), row_out, dict(
        idxlo=idxlo, idxhi=idxhi,
        eattr=eattr.astype(BF), indw=indw.astype(BF),
        maskpw=maskpw, h0b=h0f.astype(BF))


def _prepare(node_ids, node_depth, edge_index, edge_attr, node_type_emb,
             depth_emb, We, be, W1, b1, gamma, beta, W2, b2, eps_param):
    key, row_out, per = _host_prep(node_ids, node_depth, edge_index, edge_attr,
                                   node_type_emb, depth_emb)
    nc = _build(key)

    we_aug = np.concatenate([np.asarray(We, np.float32),
                             np.asarray(be, np.float32)[:, None, :]], axis=1)
    common = {
        "welb": we_aug.astype(BF),
        "w1": np.asarray(W1, np.float32).astype(BF),
        "w2": np.asarray(W2, np.float32).astype(BF),
        "gamma": np.asarray(gamma, np.float32).reshape(L, HID, 1),
        "beta": np.asarray(beta, np.float32).reshape(L, HID, 1),
        "b2b": np.asarray(b2, np.float32).reshape(L, 1, EMB).astype(BF),
        "epsr": np.broadcast_to(
            (1.0 + np.asarray(eps_param, np.float32))[:, None, None],
            (L, 128, 1)).copy(),
    }
    in_maps = []
    for c in range(NCORES):
        m = dict(common)
        for k in ("idxlo", "idxhi", "eattr", "indw", "maskpw", "h0b"):
            m[k] = per[k][c]
        in_maps.append(m)
    return nc, in_maps, row_out


def _assemble(res, row_out):
    full = np.concatenate([res.results[c]["out"] for c in range(NCORES)], axis=0)
    return full[row_out].astype(np.float32)


def kernel(**inputs):
    nc, in_maps, row_out = _prepare(**inputs)
    res = bass_utils.run_bass_kernel_spmd(nc, in_maps, core_ids=list(range(NCORES)))
    return _assemble(res, row_out)
